# revision 6
# baseline (speedup 1.0000x reference)
"""KoLeo loss kernel for Trainium2 (8 NeuronCores) — circulant-triangle fp8
gram (5/8 of the full similarity matrix per symmetry), host-normalized.

loss = -mean_i log( || xn_i - xn_{nn(i)} ||_2 + eps ),  xn = row-normalized x,
nn(i) = argmax_{j != i} xn_i . xn_j.  For unit rows ||xn_i - xn_j||^2 =
2 - 2 sim_ij, so only the row MAX of sim (diag excluded) is needed — and sim
is SYMMETRIC, so each unordered pair only has to be computed once.

Sharding (circulant over 8 row groups of 1024): core c computes blocks
(rows G_c x cols G_{c+d}) for d = 0..4 only.  Row maxes for G_c over
columns G_{c}..G_{c+4} come from the row direction (racc); columns of the
d=1..3 blocks are reduced across partitions (PE transpose + segmented
reduce) into per-group column-max partials (cacc) that cover the mirror
pairs; d=4 blocks are computed by both endpoint cores, so they need no
mirror.  The host combines the 8 cores' partials (gather + elementwise max
+ log + mean, ~100us of numpy on 32KB/core).

Host normalizes x in fp32 BEFORE fp8(e4m3) quantization, so the gram of the
quantized rows IS the similarity; no on-device norms or scales (coverage +
accuracy of this exact scheme verified against the fp32 reference in numpy:
rel err 9.8e-5, gate 2e-2).  The diagonal is suppressed in the PE: one
extra DoubleRow matmul per d=0 tile accumulates -2*I into PSUM.

Device program per core (identical SPMD, data rotated so own rows sit at
columns 0..1023):
- PE: warmup burst (p-state ramp), fp8 DR gram: 8 x K=256 matmuls per
  [128,1024] wide PSUM tile (3-buf rotation over 6 banks), -2I diag fixes,
  24 bf16 transposes for the cross-partition reduction.
- ACT: one wide Copy PSUM->bf16 SBUF per tile.
- DVE: per tile 2 narrow 2x tensor_max into per-m row accumulators; chained
  2x folds of the d=1..3 tiles into per-group column accumulators;
  segmented reduces for the transposed column maxes and the final row maxes.
- Output: [128, 32] f32 per core (8 own-row maxes + 3x8 column-max
  partials); the scalar loss is assembled on the host in float64.

Cost-model 53668ns/core (HW-verified rel err 9.64e-05); previous fp8
DoubleRow full-gram kernel 121388ns, original bf16 kernel 239308ns.

Hardware-verified constraints that shaped this (micro-tested on trn2):
tensor_mask_reduce / tensor_tensor_reduce crash the exec unit (the whole
custom-DVE reduce family is unusable); gpsimd tensor max crashes but
sub/relu/add work (3-op max emulation is correct yet Pool is ~4x too slow
to help); 2-bank [128,1024] PSUM access patterns, DVE reduce_max straight
from PSUM, PE transpose with bf16 PSUM output, and fp8 -2I accumulate
matmuls (start=False second group) all work.
"""

import os
import sys

import numpy as np

for _p in ("/opt/trn_rl_repo", "/root/.axon_site/_ro/trn_rl_repo"):
    if os.path.isdir(_p) and _p not in sys.path:
        sys.path.insert(0, _p)

import ml_dtypes  # noqa: E402
from contextlib import ExitStack  # noqa: E402

import concourse.bass as bass  # noqa: E402
import concourse.tile as tile  # noqa: E402
from concourse import bacc, mybir  # noqa: E402
from concourse.bass_utils import run_bass_kernel_spmd  # noqa: E402

N = 8192          # rows
D = 1024          # features
NCORES = 8
R = N // NCORES   # rows per core (1024)
CH = 512          # column chunk
SK = D // 256     # 4 super-k tiles (256 features = 2 x 128 for DoubleRow)
MT = R // 128     # 8 own-row blocks of 128
W = 2 * CH        # wide tile width (2 PSUM banks)
ND = 5            # groups d = 0..4
NPL = ND          # pair tiles loaded (local col groups 0..4)

F32 = mybir.dt.float32
BF16 = mybir.dt.bfloat16
FP8 = mybir.dt.float8e4
AF = mybir.ActivationFunctionType
AX = mybir.AxisListType
DR = mybir.MatmulPerfMode.DoubleRow

WARMUP = int(os.environ.get("KOLEO_WARMUP", "55"))
NDIR = int(os.environ.get("KOLEO_NDIR", "0"))
D0MODE = int(os.environ.get("KOLEO_D0", "8"))   # 0=none, 1=all, 2=even m
POOLD1 = os.environ.get("KOLEO_POOLD1", "0") == "1"  # d1 cacc folds on Pool
# mirror groups fold to FOLDK sub-accumulators on device (0 = fold to 1 +
# on-device cross-partition transpose/reduce); K>0 ships the K bf16 tiles
# per group and the host combine takes the last 128-partition max
FOLDK = int(os.environ.get("KOLEO_FOLDK", "4"))
FSPLIT = os.environ.get("KOLEO_FSPLIT", "1") == "1"  # split racc final in halves
SUPER = os.environ.get("KOLEO_SUPER", "0") == "1"  # 4-bank [128,2048] super-tiles

_CACHE = {}


def _build_program():
    from concourse.alu_op_type import AluOpType

    nc = bacc.Bacc("TRN2", target_bir_lowering=False, debug=False,
                   num_devices=NCORES)

    xq = nc.dram_tensor("xq", [NPL * 128, 2 * SK * 1024], FP8,
                        kind="ExternalInput").ap()
    # cols 0..7: own-row maxes (m); cols 8 + 8*(d-1) + b: column-max partial
    # for column 128b+p of group c+d, d in {1,2,3}
    outv = nc.dram_tensor("outv", [128, 32], F32, kind="ExternalOutput").ap()
    caccv = None
    if FOLDK:
        caccv = nc.dram_tensor("caccv", [128, 3 * FOLDK * 1024], BF16,
                               kind="ExternalOutput").ap()

    eye8 = np.eye(128, dtype=ml_dtypes.float8_e4m3)
    idpos_d = nc.inline_tensor(np.stack([eye8, eye8], axis=1), "idpos")
    idneg_d = nc.inline_tensor(np.stack([-eye8, -eye8], axis=1), "idneg")
    identb_d = nc.inline_tensor(np.eye(128, dtype=ml_dtypes.bfloat16), "identb")

    with tile.TileContext(nc) as tc, ExitStack() as ctx:
        const_pool = ctx.enter_context(tc.tile_pool(name="const", bufs=1))
        xq_pool = ctx.enter_context(tc.tile_pool(name="xqstage", bufs=1))
        cp_pool = ctx.enter_context(tc.tile_pool(name="cp", bufs=int(os.environ.get("KOLEO_CP", "8"))))
        pl_pool = ctx.enter_context(tc.tile_pool(name="pl", bufs=2))
        acc_pool = ctx.enter_context(tc.tile_pool(name="acc", bufs=1))
        stat_pool = ctx.enter_context(tc.tile_pool(name="stat", bufs=1))
        if FOLDK:
            # no on-device crosspart: all 8 banks go to the gram rotation
            # (the warmup burst rides rotation slot 0)
            ps_s = ctx.enter_context(tc.tile_pool(name="psS",
                                                  bufs=2 if SUPER else 4,
                                                  space="PSUM"))
            ps_w = ps_t = None
        else:
            ps_w = ctx.enter_context(tc.tile_pool(name="psw", bufs=1,
                                                  space="PSUM"))
            ps_t = ctx.enter_context(tc.tile_pool(name="pst", bufs=1,
                                                  space="PSUM"))
            ps_s = ctx.enter_context(tc.tile_pool(name="psS", bufs=3,
                                                  space="PSUM"))

        # warm tile: uninitialized SBUF (contents irrelevant — the warmup
        # matmuls only spin the PE p-state; their PSUM output is dead)
        warm = const_pool.tile([128, 2, 128], FP8, tag="warm")
        if os.environ.get("KOLEO_WMEMSET", "1") == "1":
            nc.vector.memset(warm[:], 0.015625)
        if FOLDK:
            warm_full = ps_s.tile([128, 2 * W if SUPER else W], F32,
                                  tag="s_ps")
            warm_ps = warm_full[:, 0:128]
        else:
            warm_ps = ps_w.tile([128, 128], F32, tag="wps")[:]
        for _ in range(WARMUP):
            nc.tensor.matmul(warm_ps, warm[:], warm[:],
                             start=True, stop=True, perf_mode=DR)

        # preload the ACT Copy table (gates the first drain)
        pre = stat_pool.tile([128, 1], F32, tag="pre")
        nc.vector.memset(pre[:], 1.0)
        nc.scalar.activation(pre[:], pre[:], AF.Copy)

        # ---- DMAs on the SP queue, first-needed first ----
        # tiny diag consts first: they gate the first (diag) tile's last
        # matmul; then pair 0 halves on the two HWDGE queues
        xp = [None] * NPL
        t0 = xq_pool.tile([128, 2 * SK, 2, CH], FP8, tag="xp0")
        # pair 0 halves on the two HWDGE queues (SP + ACT) in parallel: it
        # alone gates the first gram tile
        nc.sync.dma_start(t0[:, 0:SK], xq[0:128, 0:SK * 1024])
        nc.scalar.dma_start(t0[:, SK:2 * SK], xq[0:128, SK * 1024:])
        xp[0] = t0
        idpos = const_pool.tile([128, 2, 128], FP8, tag="idpos")
        nc.sync.dma_start(idpos[:], idpos_d[:, :, :])
        idneg = const_pool.tile([128, 2, 128], FP8, tag="idneg")
        nc.sync.dma_start(idneg[:], idneg_d[:, :, :])
        identb = const_pool.tile([128, 128], BF16, tag="identb")
        nc.scalar.dma_start(identb[:], identb_d[:, :])
        _dord = os.environ.get("KOLEO_DORD", "01234")
        for w in [int(ch) for ch in _dord if ch != "0"]:
            t = xq_pool.tile([128, 2 * SK, 2, CH], FP8, tag=f"xp{w}")
            nc.sync.dma_start(t[:], xq[w * 128:(w + 1) * 128, :])
            xp[w] = t

        def xv(n, sk):
            """[128, 2, CH] DoubleRow view of local chunk n, super-k sk."""
            return xp[n // 2][:, (n % 2) * SK + sk, :, :]

        # row-max accumulators (narrow, halves final reduce volume) + output
        outt = stat_pool.tile([128, 32], F32, tag="outt")
        nc.vector.memset(outt[:], -3.0)
        slota = stat_pool.tile([128, MT], F32, tag="slota")
        slotb = stat_pool.tile([128, MT], F32, tag="slotb")
        nc.vector.memset(slotb[:], -3.0)
        slotc = stat_pool.tile([128, MT], F32, tag="slotc")
        nc.vector.memset(slotc[:], -3.0)
        racc_all = acc_pool.tile([128, MT, CH], BF16, tag="racc_all")
        racc = [None] * MT
        if FOLDK:
            cacc = [[None] * FOLDK for _ in range(3)]
        else:
            cacc = [None] * 3        # per-group column accumulators, d=1..3
        cacc_n = [0] * 3

        def emit_crosspart(d):
            """Transpose cacc[d-1] and reduce across original partitions."""
            tp = ps_t.tile([128, MT, 128], BF16, tag="tp")
            for b in range(MT):
                nc.tensor.transpose(tp[:, b], cacc[d - 1][:, b * 128:(b + 1) * 128],
                                    identb[:])
            nc.vector.reduce_max(outt[:, 8 * d:8 * d + 8], tp[:], axis=AX.X)

        dord = os.environ.get("KOLEO_DORD", "01234")
        sched = [(int(ch), m) for ch in dord for m in range(MT)]
        # d=4 contributions must precede each m's final reduce: emit finals
        # only after every tile of that m is drained
        seen = {}
        for d, m in sched:
            seen[m] = seen.get(m, 0) + 1
        done_m = {}
        emitted = set()
        nfin = [0]

        def _is_d0_direct(d, m):
            return d == 0 and ((D0MODE == 1) or (D0MODE == 2 and m % 2 == 0)
                               or (D0MODE == 7 and m % 3 == 0)
                               or (D0MODE == 8 and m % 2 == 1)
                               or (3 <= D0MODE <= 6 and m < D0MODE))

        NSPLIT = int(os.environ.get("KOLEO_NSPLIT", "0"))  # d0 m<NSPLIT h-split
        n_direct = sum(1 for d, m in sched
                       if _is_d0_direct(d, m) or (d == 0 and m < NSPLIT))
        last_act = {}
        for i, (d, m) in enumerate(sched):
            if (not _is_d0_direct(d, m) and not (d == 0 and m < NSPLIT)
                    and not (d == ND - 1 and m >= MT - NDIR)):
                last_act[m] = i

        # early narrow tiles: (d0, m, h) for m < NSPLIT, h0 gated only by
        # the first pair-0 half.  h0's ACT copy writes the racc accumulator
        # directly (no DVE); h1 is a narrow cps + one narrow DVE max.
        def emit_narrow_d0(m, h):
            s_ps = ps_s.tile([128, W], F32)
            dst = s_ps[:, 0:CH]
            for sk in range(SK):
                nc.tensor.matmul(dst, xv(m // 4, sk)[:, :, (m % 4) * 128:
                                                     (m % 4) * 128 + 128],
                                 xv(h, sk),
                                 start=(sk == 0), stop=(sk == SK - 1),
                                 perf_mode=DR)
            if h == 0:
                off = 128 * m
                nc.tensor.matmul(s_ps[:, off:off + 128], idpos[:], idneg[:],
                                 start=False, stop=True, perf_mode=DR)
                racc[m] = racc_all[:, m]
                nc.scalar.activation(racc[m][:], dst, AF.Copy)
            else:
                cpn = cp_pool.tile([128, CH], BF16, tag="cpn", bufs=4)
                nc.scalar.activation(cpn[:], dst, AF.Copy)
                nc.vector.tensor_tensor(racc[m][:], racc[m][:], cpn[:],
                                        op=AluOpType.max)

        for h in range(2):
            for m in range(NSPLIT):
                emit_narrow_d0(m, h)

        if SUPER:
            assert FOLDK == 4 and NSPLIT == 0 and NDIR == 0
            cacc2 = [None] * 3
            for kk in range(3):
                ca2 = acc_pool.tile([128, FOLDK * 1024], BF16, tag=f"cacc2_{kk}")
                cacc2[kk] = ca2
            for d in range(ND):
                for mp in range(MT // 2):
                    m_a, m_b = 2 * mp, 2 * mp + 1
                    s2 = ps_s.tile([128, 2 * W], F32, tag="s_ps")
                    for half, m in ((0, m_a), (1, m_b)):
                        base = half * W
                        for h in range(2):
                            n = 2 * d + h
                            dst = s2[:, base + h * CH:base + (h + 1) * CH]
                            for sk in range(SK):
                                nc.tensor.matmul(
                                    dst,
                                    xv(m // 4, sk)[:, :, (m % 4) * 128:
                                                   (m % 4) * 128 + 128],
                                    xv(n, sk), start=(sk == 0),
                                    stop=(sk == SK - 1), perf_mode=DR)
                        if d == 0:
                            off = base + 128 * m
                            nc.tensor.matmul(s2[:, off:off + 128], idpos[:],
                                             idneg[:], start=False, stop=True,
                                             perf_mode=DR)
                    if d == 0:
                        # even half ACT-copied into racc path, odd half direct
                        cpa = cp_pool.tile([128, W], BF16, tag="cps", bufs=8)
                        nc.scalar.activation(cpa[:], s2[:, 0:W], AF.Copy)
                        racc[m_a] = racc_all[:, m_a]
                        nc.vector.tensor_tensor(racc[m_a][:], cpa[:, 0:CH],
                                                cpa[:, CH:W], op=AluOpType.max)
                        nc.vector.reduce_max(slotb[:, m_b:m_b + 1],
                                             s2[:, W:2 * W], axis=AX.X)
                        continue
                    k = d - 1
                    if d in (1, 2, 3) and mp < 2:
                        # seed: copy straight into the cacc pair slice
                        cp2 = cacc2[k][:, m_a * 1024:(m_b + 1) * 1024]
                    else:
                        cp2t = cp_pool.tile([128, 2 * W], BF16, tag="cps2",
                                            bufs=4)
                        cp2 = cp2t[:]
                    nc.scalar.activation(cp2, s2[:], AF.Copy)
                    for half, m in ((0, m_a), (1, m_b)):
                        base = half * W
                        if racc[m] is None:
                            racc[m] = racc_all[:, m]
                            nc.vector.tensor_tensor(
                                racc[m][:], cp2[:, base:base + CH],
                                cp2[:, base + CH:base + W], op=AluOpType.max)
                        else:
                            nc.vector.tensor_tensor(
                                racc[m][:], racc[m][:],
                                cp2[:, base:base + CH], op=AluOpType.max)
                            nc.vector.tensor_tensor(
                                racc[m][:], racc[m][:],
                                cp2[:, base + CH:base + W], op=AluOpType.max)
                    if d in (1, 2, 3) and mp >= 2:
                        ja = m_a % FOLDK
                        dstc = cacc2[k][:, ja * 1024:(ja + 2) * 1024]
                        nc.vector.tensor_tensor(dstc, dstc, cp2,
                                                op=AluOpType.max)
                    if d in (1, 2, 3) and mp == MT // 2 - 1:
                        nc.sync.dma_start(
                            caccv[0:128, k * FOLDK * 1024:
                                  (k + 1) * FOLDK * 1024], cacc2[k][:])
                    if d == ND - 1:
                        if FSPLIT and mp == 1:
                            nc.vector.reduce_max(slota[:, 0:4],
                                                 racc_all[:, 0:4], axis=AX.X)
                        if mp == MT // 2 - 1:
                            nc.vector.reduce_max(slota[:, 4:MT],
                                                 racc_all[:, 4:MT], axis=AX.X)
        for pos, (d, m) in enumerate(sched):
            if SUPER:
                break
            if d == 0 and m < NSPLIT:
                continue
            if True:
                s_ps = ps_s.tile([128, W], F32)
                for h in range(2):
                    n = 2 * d + h
                    dst = s_ps[:, h * CH:(h + 1) * CH]
                    for sk in range(SK):
                        nc.tensor.matmul(dst, xv(m // 4, sk)[:, :, (m % 4) * 128:
                                                             (m % 4) * 128 + 128],
                                         xv(n, sk),
                                         start=(sk == 0), stop=(sk == SK - 1),
                                         perf_mode=DR)
                if d == 0:
                    # suppress the diagonal: accumulate -2*I onto the own
                    # 128x128 sub-block (start=False keeps PSUM contents)
                    off = 128 * m
                    nc.tensor.matmul(s_ps[:, off:off + 128], idpos[:], idneg[:],
                                     start=False, stop=True, perf_mode=DR)
                if _is_d0_direct(d, m):
                    nc.vector.reduce_max(slotb[:, m:m + 1], s_ps[:], axis=AX.X)
                    done_m[m] = done_m.get(m, 0) + 1
                    continue
                if d == ND - 1 and m >= MT - NDIR:
                    # trailing direct path: reduce straight from PSUM into
                    # slot C (this m's racc final fired at its last ACT tile)
                    nc.vector.reduce_max(slotc[:, m:m + 1], s_ps[:], axis=AX.X)
                    continue
                if (FOLDK and d in (1, 2, 3)
                        and os.environ.get("KOLEO_SEED", "1") == "1"
                        and cacc_n[d - 1] < FOLDK
                        and cacc[d - 1][cacc_n[d - 1]] is None):
                    cps = acc_pool.tile([128, W], BF16,
                                        tag=f"cacc{d}_{cacc_n[d - 1]}")
                    cacc[d - 1][cacc_n[d - 1]] = cps
                    seeded = True
                else:
                    cps = cp_pool.tile([128, W], BF16, tag="cps",
                                       bufs=int(os.environ.get("KOLEO_CP", "8")))
                    seeded = False
                nc.scalar.activation(cps[:], s_ps[:], AF.Copy)
                # row accumulate (narrow pair)
                if racc[m] is None:
                    racc[m] = racc_all[:, m]
                    nc.vector.tensor_tensor(racc[m][:], cps[:, 0:CH],
                                            cps[:, CH:W], op=AluOpType.max)
                else:
                    nc.vector.tensor_tensor(racc[m][:], racc[m][:], cps[:, 0:CH],
                                            op=AluOpType.max)
                    nc.vector.tensor_tensor(racc[m][:], racc[m][:], cps[:, CH:W],
                                            op=AluOpType.max)
                # column accumulate for mirror groups
                if d in (1, 2, 3):
                    k = d - 1
                    if FOLDK:
                        # fold into FOLDK sub-accumulators; the first FOLDK
                        # tiles of the group seed them via a DVE copy
                        j = cacc_n[k] % FOLDK
                        if seeded:
                            pass
                        elif cacc[k][j] is None:
                            ca = acc_pool.tile([128, W], BF16,
                                               tag=f"cacc{d}_{j}")
                            cacc[k][j] = ca
                            nc.vector.tensor_copy(ca[:], cps[:])
                        else:
                            nc.vector.tensor_tensor(cacc[k][j][:],
                                                    cacc[k][j][:], cps[:],
                                                    op=AluOpType.max)
                        cacc_n[k] += 1
                        if cacc_n[k] == MT:
                            for j2 in range(FOLDK):
                                nc.sync.dma_start(
                                    caccv[0:128,
                                          (k * FOLDK + j2) * 1024:
                                          (k * FOLDK + j2 + 1) * 1024],
                                    cacc[k][j2][:])
                    else:
                        if cacc_n[k] == 0:
                            pend_attr = f"_pend{k}"
                            setattr(emit_crosspart, pend_attr, cps)
                        elif cacc_n[k] == 1:
                            ca = acc_pool.tile([128, W], BF16, tag=f"cacc{d}")
                            cacc[k] = ca
                            prev = getattr(emit_crosspart, f"_pend{k}")
                            nc.vector.tensor_tensor(ca[:], prev[:], cps[:],
                                                    op=AluOpType.max)
                        else:
                            nc.vector.tensor_tensor(cacc[k][:], cacc[k][:],
                                                    cps[:], op=AluOpType.max)
                        cacc_n[k] += 1
                        if cacc_n[k] == MT:
                            emit_crosspart(d)
                # row finals: segmented reduce(s) over racc_all
                if FSPLIT:
                    if os.environ.get("KOLEO_FQ", "1") == "1":
                        # quarters: fire after each odd-m d4 tile
                        if d == ND - 1 and m % 2 == 1:
                            q = m - 1
                            nc.vector.reduce_max(slota[:, q:q + 2],
                                                 racc_all[:, q:q + 2],
                                                 axis=AX.X)
                    else:
                        # half 1 once m0..3's racc is complete (after (4,3)),
                        # half 2 after the last ACT-path tile
                        if (d, m) == (ND - 1, 3):
                            nc.vector.reduce_max(slota[:, 0:4],
                                                 racc_all[:, 0:4], axis=AX.X)
                        if pos == max(last_act.values()):
                            nc.vector.reduce_max(slota[:, 4:MT],
                                                 racc_all[:, 4:MT], axis=AX.X)
                else:
                    nfin[0] += 1
                    if nfin[0] == len(sched) - n_direct - NSPLIT:
                        nc.vector.reduce_max(slota[:], racc_all[:], axis=AX.X)

        nc.vector.tensor_tensor(outt[:, 0:MT], slota[:], slotb[:],
                                op=AluOpType.max)
        nc.vector.tensor_tensor(outt[:, 0:MT], outt[:, 0:MT], slotc[:],
                                op=AluOpType.max)
        nc.sync.dma_start(outv[:], outt[:])

    nc.compile()
    return nc


def _prep_core_input(xn8: np.ndarray, core: int) -> np.ndarray:
    """Transpose + rotate + DoubleRow-interleave pre-quantized rows; keep
    only the NPL local column groups this core consumes."""
    s = core * R
    rolled = np.concatenate([xn8[s:], xn8[:s]], axis=0)[:NPL * R]  # [NPL*R, D]
    xq8 = np.ascontiguousarray(rolled.T)                           # [D, NPL*R]
    h = xq8.reshape(SK, 2, 128, NPL, 2, CH).transpose(3, 2, 4, 0, 1, 5)
    return np.ascontiguousarray(h.reshape(NPL * 128, 2 * SK * 1024))


def _run(student_output: np.ndarray, **spmd_kwargs):
    x = np.asarray(student_output, dtype=np.float32)
    assert x.shape == (N, D), x.shape

    if "nc" not in _CACHE:
        _CACHE["nc"] = _build_program()
    nc = _CACHE["nc"]

    norm = np.sqrt(np.sum(x.astype(np.float64) ** 2, axis=1, keepdims=True))
    xn = (x / np.maximum(norm, 1e-8)).astype(np.float32)
    xn8 = xn.astype(ml_dtypes.float8_e4m3)

    in_maps = [{"xq": _prep_core_input(xn8, c)} for c in range(NCORES)]

    res = None
    for attempt in range(3):
        try:
            res = run_bass_kernel_spmd(nc, in_maps, list(range(NCORES)),
                                       **spmd_kwargs)
            break
        except Exception:
            # transient NRT_EXEC_UNIT_UNRECOVERABLE under axon; retry fresh
            if attempt == 2:
                raise
            import time

            try:
                import jax

                jax.clear_caches()
                jax.extend.backend.clear_backends()
            except Exception:
                pass
            time.sleep(5.0)

    # host combine: gather/unshard the 8 partial-max vectors, final formula
    final = np.full(N, -3.0, np.float64)
    for c in range(NCORES):
        ov = np.asarray(res.results[c]["outv"], np.float64)  # [128, 32]
        rows = slice(c * R, (c + 1) * R)
        final[rows] = np.maximum(final[rows], ov[:, 0:MT].T.reshape(R))
        if FOLDK:
            cv = np.asarray(res.results[c]["caccv"], np.float64)
            cv = cv.reshape(128, 3, FOLDK, 1024)
            colmax = cv.max(axis=(0, 2))                      # [3, 1024]
            for d in (1, 2, 3):
                g = (c + d) % NCORES
                rows_g = slice(g * R, (g + 1) * R)
                final[rows_g] = np.maximum(final[rows_g], colmax[d - 1])
        else:
            for d in (1, 2, 3):
                g = (c + d) % NCORES
                rows_g = slice(g * R, (g + 1) * R)
                final[rows_g] = np.maximum(final[rows_g],
                                           ov[:, 8 * d:8 * d + 8].T.reshape(R))
    s = np.minimum(final, 1.0 - 1e-7)
    dist = np.sqrt(np.maximum(2.0 - 2.0 * s, 0.0))
    loss = -np.mean(np.log(dist + 1e-8))
    return np.asarray(loss, dtype=np.float32), res


def kernel(student_output: np.ndarray) -> np.ndarray:
    return _run(student_output)[0]


# revision 7
# speedup vs baseline: 1.0013x; 1.0013x over previous
"""KoLeo loss kernel for Trainium2 (8 NeuronCores) — circulant-triangle fp8
gram (5/8 of the full similarity matrix per symmetry), host-normalized.

loss = -mean_i log( || xn_i - xn_{nn(i)} ||_2 + eps ),  xn = row-normalized x,
nn(i) = argmax_{j != i} xn_i . xn_j.  For unit rows ||xn_i - xn_j||^2 =
2 - 2 sim_ij, so only the row MAX of sim (diag excluded) is needed — and sim
is SYMMETRIC, so each unordered pair only has to be computed once.

Sharding (circulant over 8 row groups of 1024): core c computes blocks
(rows G_c x cols G_{c+d}) for d = 0..4 only.  Row maxes for G_c over
columns G_{c}..G_{c+4} come from the row direction (racc); columns of the
d=1..3 blocks are reduced across partitions (PE transpose + segmented
reduce) into per-group column-max partials (cacc) that cover the mirror
pairs; d=4 blocks are computed by both endpoint cores, so they need no
mirror.  The host combines the 8 cores' partials (gather + elementwise max
+ log + mean, ~100us of numpy on 32KB/core).

Host normalizes x in fp32 BEFORE fp8(e4m3) quantization, so the gram of the
quantized rows IS the similarity; no on-device norms or scales (coverage +
accuracy of this exact scheme verified against the fp32 reference in numpy:
rel err 9.8e-5, gate 2e-2).  The diagonal is suppressed in the PE: one
extra DoubleRow matmul per d=0 tile accumulates -2*I into PSUM.

Device program per core (identical SPMD, data rotated so own rows sit at
columns 0..1023):
- PE: warmup burst (p-state ramp), fp8 DR gram: 8 x K=256 matmuls per
  [128,1024] wide PSUM tile (3-buf rotation over 6 banks), -2I diag fixes,
  24 bf16 transposes for the cross-partition reduction.
- ACT: one wide Copy PSUM->bf16 SBUF per tile.
- DVE: per tile 2 narrow 2x tensor_max into per-m row accumulators; chained
  2x folds of the d=1..3 tiles into per-group column accumulators;
  segmented reduces for the transposed column maxes and the final row maxes.
- Output: [128, 32] f32 per core (8 own-row maxes + 3x8 column-max
  partials); the scalar loss is assembled on the host in float64.

Cost-model 53504ns/core (HW-verified rel err 9.64e-05); previous fp8
DoubleRow full-gram kernel 121388ns, original bf16 kernel 239308ns.

Hardware-verified constraints that shaped this (micro-tested on trn2):
tensor_mask_reduce / tensor_tensor_reduce crash the exec unit (the whole
custom-DVE reduce family is unusable); gpsimd tensor max crashes but
sub/relu/add work (3-op max emulation is correct yet Pool is ~4x too slow
to help); 2-bank [128,1024] PSUM access patterns, DVE reduce_max straight
from PSUM, PE transpose with bf16 PSUM output, and fp8 -2I accumulate
matmuls (start=False second group) all work.
"""

import os
import sys

import numpy as np

for _p in ("/opt/trn_rl_repo", "/root/.axon_site/_ro/trn_rl_repo"):
    if os.path.isdir(_p) and _p not in sys.path:
        sys.path.insert(0, _p)

import ml_dtypes  # noqa: E402
from contextlib import ExitStack  # noqa: E402

import concourse.bass as bass  # noqa: E402
import concourse.tile as tile  # noqa: E402
from concourse import bacc, mybir  # noqa: E402
from concourse.bass_utils import run_bass_kernel_spmd  # noqa: E402

N = 8192          # rows
D = 1024          # features
NCORES = 8
R = N // NCORES   # rows per core (1024)
CH = 512          # column chunk
SK = D // 256     # 4 super-k tiles (256 features = 2 x 128 for DoubleRow)
MT = R // 128     # 8 own-row blocks of 128
W = 2 * CH        # wide tile width (2 PSUM banks)
ND = 5            # groups d = 0..4
NPL = ND          # pair tiles loaded (local col groups 0..4)

F32 = mybir.dt.float32
BF16 = mybir.dt.bfloat16
FP8 = mybir.dt.float8e4
AF = mybir.ActivationFunctionType
AX = mybir.AxisListType
DR = mybir.MatmulPerfMode.DoubleRow

WARMUP = int(os.environ.get("KOLEO_WARMUP", "55"))
NDIR = int(os.environ.get("KOLEO_NDIR", "0"))
D0MODE = int(os.environ.get("KOLEO_D0", "8"))   # 0=none, 1=all, 2=even m
POOLD1 = os.environ.get("KOLEO_POOLD1", "0") == "1"  # d1 cacc folds on Pool
# mirror groups fold to FOLDK sub-accumulators on device (0 = fold to 1 +
# on-device cross-partition transpose/reduce); K>0 ships the K bf16 tiles
# per group and the host combine takes the last 128-partition max
FOLDK = int(os.environ.get("KOLEO_FOLDK", "4"))
FSPLIT = os.environ.get("KOLEO_FSPLIT", "1") == "1"  # split racc final in halves
SUPER = os.environ.get("KOLEO_SUPER", "0") == "1"  # 4-bank [128,2048] super-tiles

_CACHE = {}


def _build_program():
    from concourse.alu_op_type import AluOpType

    nc = bacc.Bacc("TRN2", target_bir_lowering=False, debug=False,
                   num_devices=NCORES)

    xq = nc.dram_tensor("xq", [NPL * 128, 2 * SK * 1024], FP8,
                        kind="ExternalInput").ap()
    # cols 0..7: own-row maxes (m); cols 8 + 8*(d-1) + b: column-max partial
    # for column 128b+p of group c+d, d in {1,2,3}
    outv = nc.dram_tensor("outv", [128, 32], F32, kind="ExternalOutput").ap()
    caccv = None
    if FOLDK:
        caccv = nc.dram_tensor("caccv", [128, 3 * FOLDK * 1024], BF16,
                               kind="ExternalOutput").ap()

    eye8 = np.eye(128, dtype=ml_dtypes.float8_e4m3)
    idpos_d = nc.inline_tensor(np.stack([eye8, eye8], axis=1), "idpos")
    idneg_d = nc.inline_tensor(np.stack([-eye8, -eye8], axis=1), "idneg")
    identb_d = nc.inline_tensor(np.eye(128, dtype=ml_dtypes.bfloat16), "identb")

    with tile.TileContext(nc) as tc, ExitStack() as ctx:
        const_pool = ctx.enter_context(tc.tile_pool(name="const", bufs=1))
        xq_pool = ctx.enter_context(tc.tile_pool(name="xqstage", bufs=1))
        cp_pool = ctx.enter_context(tc.tile_pool(name="cp", bufs=int(os.environ.get("KOLEO_CP", "8"))))
        pl_pool = ctx.enter_context(tc.tile_pool(name="pl", bufs=2))
        acc_pool = ctx.enter_context(tc.tile_pool(name="acc", bufs=1))
        stat_pool = ctx.enter_context(tc.tile_pool(name="stat", bufs=1))
        if FOLDK:
            # no on-device crosspart: all 8 banks go to the gram rotation
            # (the warmup burst rides rotation slot 0)
            ps_s = ctx.enter_context(tc.tile_pool(name="psS",
                                                  bufs=2 if SUPER else 4,
                                                  space="PSUM"))
            ps_w = ps_t = None
        else:
            ps_w = ctx.enter_context(tc.tile_pool(name="psw", bufs=1,
                                                  space="PSUM"))
            ps_t = ctx.enter_context(tc.tile_pool(name="pst", bufs=1,
                                                  space="PSUM"))
            ps_s = ctx.enter_context(tc.tile_pool(name="psS", bufs=3,
                                                  space="PSUM"))

        # warm tile: uninitialized SBUF (contents irrelevant — the warmup
        # matmuls only spin the PE p-state; their PSUM output is dead)
        warm = const_pool.tile([128, 2, 128], FP8, tag="warm")
        if os.environ.get("KOLEO_WMEMSET", "1") == "1":
            nc.vector.memset(warm[:], 0.015625)
        if FOLDK:
            warm_full = ps_s.tile([128, 2 * W if SUPER else W], F32,
                                  tag="s_ps")
            warm_ps = warm_full[:, 0:128]
        else:
            warm_ps = ps_w.tile([128, 128], F32, tag="wps")[:]
        for _ in range(WARMUP):
            nc.tensor.matmul(warm_ps, warm[:], warm[:],
                             start=True, stop=True, perf_mode=DR)

        # preload the ACT Copy table (gates the first drain)
        pre = stat_pool.tile([128, 1], F32, tag="pre")
        nc.vector.memset(pre[:], 1.0)
        nc.scalar.activation(pre[:], pre[:], AF.Copy)

        # ---- DMAs on the SP queue, first-needed first ----
        # tiny diag consts first: they gate the first (diag) tile's last
        # matmul; then pair 0 halves on the two HWDGE queues
        xp = [None] * NPL
        t0 = xq_pool.tile([128, 2 * SK, 2, CH], FP8, tag="xp0")
        # pair 0 halves on the two HWDGE queues (SP + ACT) in parallel: it
        # alone gates the first gram tile
        nc.sync.dma_start(t0[:, 0:SK], xq[0:128, 0:SK * 1024])
        nc.scalar.dma_start(t0[:, SK:2 * SK], xq[0:128, SK * 1024:])
        xp[0] = t0
        idpos = const_pool.tile([128, 2, 128], FP8, tag="idpos")
        nc.sync.dma_start(idpos[:], idpos_d[:, :, :])
        idneg = const_pool.tile([128, 2, 128], FP8, tag="idneg")
        nc.sync.dma_start(idneg[:], idneg_d[:, :, :])
        identb = const_pool.tile([128, 128], BF16, tag="identb")
        nc.scalar.dma_start(identb[:], identb_d[:, :])
        _dord = os.environ.get("KOLEO_DORD", "01234")
        for w in [int(ch) for ch in _dord if ch != "0"]:
            t = xq_pool.tile([128, 2 * SK, 2, CH], FP8, tag=f"xp{w}")
            nc.sync.dma_start(t[:], xq[w * 128:(w + 1) * 128, :])
            xp[w] = t

        def xv(n, sk):
            """[128, 2, CH] DoubleRow view of local chunk n, super-k sk."""
            return xp[n // 2][:, (n % 2) * SK + sk, :, :]

        # row-max accumulators (narrow, halves final reduce volume) + output
        outt = stat_pool.tile([128, 32], F32, tag="outt")
        nc.vector.memset(outt[:], -3.0)
        slota = stat_pool.tile([128, MT], F32, tag="slota")
        slotb = stat_pool.tile([128, MT], F32, tag="slotb")
        nc.vector.memset(slotb[:], -3.0)
        slotc = stat_pool.tile([128, MT], F32, tag="slotc")
        nc.vector.memset(slotc[:], -3.0)
        racc_all = acc_pool.tile([128, MT, CH], BF16, tag="racc_all")
        racc = [None] * MT
        if FOLDK:
            cacc = [[None] * FOLDK for _ in range(3)]
        else:
            cacc = [None] * 3        # per-group column accumulators, d=1..3
        cacc_n = [0] * 3

        def emit_crosspart(d):
            """Transpose cacc[d-1] and reduce across original partitions."""
            tp = ps_t.tile([128, MT, 128], BF16, tag="tp")
            for b in range(MT):
                nc.tensor.transpose(tp[:, b], cacc[d - 1][:, b * 128:(b + 1) * 128],
                                    identb[:])
            nc.vector.reduce_max(outt[:, 8 * d:8 * d + 8], tp[:], axis=AX.X)

        dord = os.environ.get("KOLEO_DORD", "01234")
        sched = [(int(ch), m) for ch in dord for m in range(MT)]
        # d=4 contributions must precede each m's final reduce: emit finals
        # only after every tile of that m is drained
        seen = {}
        for d, m in sched:
            seen[m] = seen.get(m, 0) + 1
        done_m = {}
        emitted = set()
        nfin = [0]

        def _is_d0_direct(d, m):
            return d == 0 and ((D0MODE == 1) or (D0MODE == 2 and m % 2 == 0)
                               or (D0MODE == 7 and m % 3 == 0)
                               or (D0MODE == 8 and m % 2 == 1)
                               or (3 <= D0MODE <= 6 and m < D0MODE))

        NSPLIT = int(os.environ.get("KOLEO_NSPLIT", "0"))  # d0 m<NSPLIT h-split
        n_direct = sum(1 for d, m in sched
                       if _is_d0_direct(d, m) or (d == 0 and m < NSPLIT))
        last_act = {}
        for i, (d, m) in enumerate(sched):
            if (not _is_d0_direct(d, m) and not (d == 0 and m < NSPLIT)
                    and not (d == ND - 1 and m >= MT - NDIR)):
                last_act[m] = i

        # early narrow tiles: (d0, m, h) for m < NSPLIT, h0 gated only by
        # the first pair-0 half.  h0's ACT copy writes the racc accumulator
        # directly (no DVE); h1 is a narrow cps + one narrow DVE max.
        def emit_narrow_d0(m, h):
            s_ps = ps_s.tile([128, W], F32)
            dst = s_ps[:, 0:CH]
            for sk in range(SK):
                nc.tensor.matmul(dst, xv(m // 4, sk)[:, :, (m % 4) * 128:
                                                     (m % 4) * 128 + 128],
                                 xv(h, sk),
                                 start=(sk == 0), stop=(sk == SK - 1),
                                 perf_mode=DR)
            if h == 0:
                off = 128 * m
                nc.tensor.matmul(s_ps[:, off:off + 128], idpos[:], idneg[:],
                                 start=False, stop=True, perf_mode=DR)
                racc[m] = racc_all[:, m]
                nc.scalar.activation(racc[m][:], dst, AF.Copy)
            else:
                cpn = cp_pool.tile([128, CH], BF16, tag="cpn", bufs=4)
                nc.scalar.activation(cpn[:], dst, AF.Copy)
                nc.vector.tensor_tensor(racc[m][:], racc[m][:], cpn[:],
                                        op=AluOpType.max)

        for h in range(2):
            for m in range(NSPLIT):
                emit_narrow_d0(m, h)

        if SUPER:
            assert FOLDK == 4 and NSPLIT == 0 and NDIR == 0
            cacc2 = [None] * 3
            for kk in range(3):
                ca2 = acc_pool.tile([128, FOLDK * 1024], BF16, tag=f"cacc2_{kk}")
                cacc2[kk] = ca2
            for d in range(ND):
                for mp in range(MT // 2):
                    m_a, m_b = 2 * mp, 2 * mp + 1
                    s2 = ps_s.tile([128, 2 * W], F32, tag="s_ps")
                    for half, m in ((0, m_a), (1, m_b)):
                        base = half * W
                        for h in range(2):
                            n = 2 * d + h
                            dst = s2[:, base + h * CH:base + (h + 1) * CH]
                            for sk in range(SK):
                                nc.tensor.matmul(
                                    dst,
                                    xv(m // 4, sk)[:, :, (m % 4) * 128:
                                                   (m % 4) * 128 + 128],
                                    xv(n, sk), start=(sk == 0),
                                    stop=(sk == SK - 1), perf_mode=DR)
                        if d == 0:
                            off = base + 128 * m
                            nc.tensor.matmul(s2[:, off:off + 128], idpos[:],
                                             idneg[:], start=False, stop=True,
                                             perf_mode=DR)
                    if d == 0:
                        # even half ACT-copied into racc path, odd half direct
                        cpa = cp_pool.tile([128, W], BF16, tag="cps", bufs=8)
                        nc.scalar.activation(cpa[:], s2[:, 0:W], AF.Copy)
                        racc[m_a] = racc_all[:, m_a]
                        nc.vector.tensor_tensor(racc[m_a][:], cpa[:, 0:CH],
                                                cpa[:, CH:W], op=AluOpType.max)
                        nc.vector.reduce_max(slotb[:, m_b:m_b + 1],
                                             s2[:, W:2 * W], axis=AX.X)
                        continue
                    k = d - 1
                    if d in (1, 2, 3) and mp < 2:
                        # seed: copy straight into the cacc pair slice
                        cp2 = cacc2[k][:, m_a * 1024:(m_b + 1) * 1024]
                    else:
                        cp2t = cp_pool.tile([128, 2 * W], BF16, tag="cps2",
                                            bufs=4)
                        cp2 = cp2t[:]
                    nc.scalar.activation(cp2, s2[:], AF.Copy)
                    for half, m in ((0, m_a), (1, m_b)):
                        base = half * W
                        if racc[m] is None:
                            racc[m] = racc_all[:, m]
                            nc.vector.tensor_tensor(
                                racc[m][:], cp2[:, base:base + CH],
                                cp2[:, base + CH:base + W], op=AluOpType.max)
                        else:
                            nc.vector.tensor_tensor(
                                racc[m][:], racc[m][:],
                                cp2[:, base:base + CH], op=AluOpType.max)
                            nc.vector.tensor_tensor(
                                racc[m][:], racc[m][:],
                                cp2[:, base + CH:base + W], op=AluOpType.max)
                    if d in (1, 2, 3) and mp >= 2:
                        ja = m_a % FOLDK
                        dstc = cacc2[k][:, ja * 1024:(ja + 2) * 1024]
                        nc.vector.tensor_tensor(dstc, dstc, cp2,
                                                op=AluOpType.max)
                    if d in (1, 2, 3) and mp == MT // 2 - 1:
                        nc.sync.dma_start(
                            caccv[0:128, k * FOLDK * 1024:
                                  (k + 1) * FOLDK * 1024], cacc2[k][:])
                    if d == ND - 1:
                        if FSPLIT and mp == 1:
                            nc.vector.reduce_max(slota[:, 0:4],
                                                 racc_all[:, 0:4], axis=AX.X)
                        if mp == MT // 2 - 1:
                            nc.vector.reduce_max(slota[:, 4:MT],
                                                 racc_all[:, 4:MT], axis=AX.X)
        for pos, (d, m) in enumerate(sched):
            if SUPER:
                break
            if d == 0 and m < NSPLIT:
                continue
            if True:
                s_ps = ps_s.tile([128, W], F32)
                for h in range(2):
                    n = 2 * d + h
                    dst = s_ps[:, h * CH:(h + 1) * CH]
                    for sk in range(SK):
                        nc.tensor.matmul(dst, xv(m // 4, sk)[:, :, (m % 4) * 128:
                                                             (m % 4) * 128 + 128],
                                         xv(n, sk),
                                         start=(sk == 0), stop=(sk == SK - 1),
                                         perf_mode=DR)
                if d == 0:
                    # suppress the diagonal: accumulate -2*I onto the own
                    # 128x128 sub-block (start=False keeps PSUM contents)
                    off = 128 * m
                    nc.tensor.matmul(s_ps[:, off:off + 128], idpos[:], idneg[:],
                                     start=False, stop=True, perf_mode=DR)
                if _is_d0_direct(d, m):
                    nc.vector.reduce_max(slotb[:, m:m + 1], s_ps[:], axis=AX.X)
                    done_m[m] = done_m.get(m, 0) + 1
                    continue
                if d == ND - 1 and m >= MT - NDIR:
                    # trailing direct path: reduce straight from PSUM into
                    # slot C (this m's racc final fired at its last ACT tile)
                    nc.vector.reduce_max(slotc[:, m:m + 1], s_ps[:], axis=AX.X)
                    continue
                if (FOLDK and d in (1, 2, 3)
                        and os.environ.get("KOLEO_SEED", "1") == "1"
                        and cacc_n[d - 1] < FOLDK
                        and cacc[d - 1][cacc_n[d - 1]] is None):
                    cps = acc_pool.tile([128, W], BF16,
                                        tag=f"cacc{d}_{cacc_n[d - 1]}")
                    cacc[d - 1][cacc_n[d - 1]] = cps
                    seeded = True
                else:
                    cps = cp_pool.tile([128, W], BF16, tag="cps",
                                       bufs=int(os.environ.get("KOLEO_CP", "8")))
                    seeded = False
                nc.scalar.activation(cps[:], s_ps[:], AF.Copy)
                # row accumulate (narrow pair)
                if racc[m] is None:
                    racc[m] = racc_all[:, m]
                    nc.vector.tensor_tensor(racc[m][:], cps[:, 0:CH],
                                            cps[:, CH:W], op=AluOpType.max)
                else:
                    nc.vector.tensor_tensor(racc[m][:], racc[m][:], cps[:, 0:CH],
                                            op=AluOpType.max)
                    nc.vector.tensor_tensor(racc[m][:], racc[m][:], cps[:, CH:W],
                                            op=AluOpType.max)
                # column accumulate for mirror groups
                if d in (1, 2, 3):
                    k = d - 1
                    if FOLDK:
                        # fold into FOLDK sub-accumulators; the first FOLDK
                        # tiles of the group seed them via a DVE copy
                        j = cacc_n[k] % FOLDK
                        if seeded:
                            pass
                        elif cacc[k][j] is None:
                            ca = acc_pool.tile([128, W], BF16,
                                               tag=f"cacc{d}_{j}")
                            cacc[k][j] = ca
                            nc.vector.tensor_copy(ca[:], cps[:])
                        else:
                            nc.vector.tensor_tensor(cacc[k][j][:],
                                                    cacc[k][j][:], cps[:],
                                                    op=AluOpType.max)
                        cacc_n[k] += 1
                        if cacc_n[k] == MT:
                            for j2 in range(FOLDK):
                                nc.sync.dma_start(
                                    caccv[0:128,
                                          (k * FOLDK + j2) * 1024:
                                          (k * FOLDK + j2 + 1) * 1024],
                                    cacc[k][j2][:])
                    else:
                        if cacc_n[k] == 0:
                            pend_attr = f"_pend{k}"
                            setattr(emit_crosspart, pend_attr, cps)
                        elif cacc_n[k] == 1:
                            ca = acc_pool.tile([128, W], BF16, tag=f"cacc{d}")
                            cacc[k] = ca
                            prev = getattr(emit_crosspart, f"_pend{k}")
                            nc.vector.tensor_tensor(ca[:], prev[:], cps[:],
                                                    op=AluOpType.max)
                        else:
                            nc.vector.tensor_tensor(cacc[k][:], cacc[k][:],
                                                    cps[:], op=AluOpType.max)
                        cacc_n[k] += 1
                        if cacc_n[k] == MT:
                            emit_crosspart(d)
                # row finals: segmented reduce(s) over racc_all
                if FSPLIT:
                    if os.environ.get("KOLEO_FQ", "1") == "8":
                        # eighths: fire after every d4 tile
                        if d == ND - 1:
                            nc.vector.reduce_max(slota[:, m:m + 1],
                                                 racc_all[:, m:m + 1],
                                                 axis=AX.X)
                    elif os.environ.get("KOLEO_FQ", "1") == "1":
                        # quarters: fire after each odd-m d4 tile
                        if d == ND - 1 and m % 2 == 1:
                            q = m - 1
                            nc.vector.reduce_max(slota[:, q:q + 2],
                                                 racc_all[:, q:q + 2],
                                                 axis=AX.X)
                    else:
                        # half 1 once m0..3's racc is complete (after (4,3)),
                        # half 2 after the last ACT-path tile
                        if (d, m) == (ND - 1, 3):
                            nc.vector.reduce_max(slota[:, 0:4],
                                                 racc_all[:, 0:4], axis=AX.X)
                        if pos == max(last_act.values()):
                            nc.vector.reduce_max(slota[:, 4:MT],
                                                 racc_all[:, 4:MT], axis=AX.X)
                else:
                    nfin[0] += 1
                    if nfin[0] == len(sched) - n_direct - NSPLIT:
                        nc.vector.reduce_max(slota[:], racc_all[:], axis=AX.X)

        nc.vector.tensor_tensor(outt[:, 0:MT], slota[:], slotb[:],
                                op=AluOpType.max)
        if NDIR > 0 or NSPLIT > 0:
            nc.vector.tensor_tensor(outt[:, 0:MT], outt[:, 0:MT], slotc[:],
                                    op=AluOpType.max)
        nc.sync.dma_start(outv[:], outt[:])

    nc.compile()
    return nc


def _prep_core_input(xn8: np.ndarray, core: int) -> np.ndarray:
    """Transpose + rotate + DoubleRow-interleave pre-quantized rows; keep
    only the NPL local column groups this core consumes."""
    s = core * R
    rolled = np.concatenate([xn8[s:], xn8[:s]], axis=0)[:NPL * R]  # [NPL*R, D]
    xq8 = np.ascontiguousarray(rolled.T)                           # [D, NPL*R]
    h = xq8.reshape(SK, 2, 128, NPL, 2, CH).transpose(3, 2, 4, 0, 1, 5)
    return np.ascontiguousarray(h.reshape(NPL * 128, 2 * SK * 1024))


def _run(student_output: np.ndarray, **spmd_kwargs):
    x = np.asarray(student_output, dtype=np.float32)
    assert x.shape == (N, D), x.shape

    if "nc" not in _CACHE:
        _CACHE["nc"] = _build_program()
    nc = _CACHE["nc"]

    norm = np.sqrt(np.sum(x.astype(np.float64) ** 2, axis=1, keepdims=True))
    xn = (x / np.maximum(norm, 1e-8)).astype(np.float32)
    xn8 = xn.astype(ml_dtypes.float8_e4m3)

    in_maps = [{"xq": _prep_core_input(xn8, c)} for c in range(NCORES)]

    res = None
    for attempt in range(3):
        try:
            res = run_bass_kernel_spmd(nc, in_maps, list(range(NCORES)),
                                       **spmd_kwargs)
            break
        except Exception:
            # transient NRT_EXEC_UNIT_UNRECOVERABLE under axon; retry fresh
            if attempt == 2:
                raise
            import time

            try:
                import jax

                jax.clear_caches()
                jax.extend.backend.clear_backends()
            except Exception:
                pass
            time.sleep(5.0)

    # host combine: gather/unshard the 8 partial-max vectors, final formula
    final = np.full(N, -3.0, np.float64)
    for c in range(NCORES):
        ov = np.asarray(res.results[c]["outv"], np.float64)  # [128, 32]
        rows = slice(c * R, (c + 1) * R)
        final[rows] = np.maximum(final[rows], ov[:, 0:MT].T.reshape(R))
        if FOLDK:
            cv = np.asarray(res.results[c]["caccv"], np.float64)
            cv = cv.reshape(128, 3, FOLDK, 1024)
            colmax = cv.max(axis=(0, 2))                      # [3, 1024]
            for d in (1, 2, 3):
                g = (c + d) % NCORES
                rows_g = slice(g * R, (g + 1) * R)
                final[rows_g] = np.maximum(final[rows_g], colmax[d - 1])
        else:
            for d in (1, 2, 3):
                g = (c + d) % NCORES
                rows_g = slice(g * R, (g + 1) * R)
                final[rows_g] = np.maximum(final[rows_g],
                                           ov[:, 8 * d:8 * d + 8].T.reshape(R))
    s = np.minimum(final, 1.0 - 1e-7)
    dist = np.sqrt(np.maximum(2.0 - 2.0 * s, 0.0))
    loss = -np.mean(np.log(dist + 1e-8))
    return np.asarray(loss, dtype=np.float32), res


def kernel(student_output: np.ndarray) -> np.ndarray:
    return _run(student_output)[0]
